# revision 36
# baseline (speedup 1.0000x reference)
"""Trainium2 Bass kernel for a (quirky) transformer decoder layer.

Problem shapes: B=2, S=2048, D=128, H=8 heads, head_dim=16.
  sa  = attn(q=x_tgt, kv=x_tgt);  r1 = sa @ w1 + b1 + x_tgt
  ca  = attn(q=enc_out, kv=x_tgt); r2 = ca @ w2 + b2 + r1
  ln  = (r2 - mean) / var   (var unbiased, divide by var not std)
  out = relu(ln @ w3 + b3) @ w4 + b4 + r2
(mask_src / mask_tgt are unused by the reference.)

Sharding: 8 cores, query-row sharding (zero communication). Core c handles
batch c//4, query rows [(c%4)*512 : (c%4+1)*512]. K/V are computed per-core
from the full 2048-row x_tgt of its batch (small replicated work).

The kernel is ScalarE-bound: softmax exp over 2 x 8 heads x 2048 x 512
scores = 16.8M elements/core = ~110us of ACT time; everything else hides
under it. Structure:
  - all matmuls bf16 (1 cycle/row + FWL weights); fp32 only for residuals,
    softmax denominators and layernorm stats;
  - x^T / enc^T arrive pre-transposed from host; Q->K fused on host into
    M_h^T = 0.25 * wq_h @ wk_h^T so scores^T = x @ (M_h @ x_q^T);
  - inner pipeline batches 2 k-tiles per issue group ([4 score MMs][2 exps]
    [4 PV MMs]) with PV lagging 2 tiles, so the exp stream never waits on PE
    and same-type matmuls issue back-to-back (the PE pays ~85ns per
    instruction-type transition);
  - setup matmuls (G heads, packed V) are woven one-per-tile across all of
    attn0's sets, alternating between two idle PSUM banks;
  - ~26 of 128 exp tiles run on the DVE instead via a one-instruction
    Schraudolph bit-trick ((sc*c1+c2) cast to int16, bit-viewed as bf16);
  - softmax denominator rows come from ones columns in the packed V,
    written by per-tile GpSimd memsets;
  - input DMAs split across the SP and ACT hardware queues + GpSimd SW DGE;
  - tail reciprocals run as Ln->Exp(-x) on the (by then idle) ScalarE;
  - output is returned transposed; host un-transposes.
"""

import ml_dtypes
import numpy as np

import concourse.bass as bass
import concourse.tile as tile
from concourse import mybir
from concourse.bass_utils import run_bass_kernel_spmd

B, S, D, H, HD = 2, 2048, 128, 8, 16
QC = 512  # query rows per core
NCORES = 8
KT = 16  # number of 128-row key tiles
F32 = mybir.dt.float32
BF16 = mybir.dt.bfloat16
I16 = mybir.dt.int16
AF = mybir.ActivationFunctionType
OP = mybir.AluOpType
NPBF = ml_dtypes.bfloat16

# Schraudolph exp in bf16 bit-space: bf16(int16(s * 2^7/ln2 + (127*2^7 - C)))
# approximates e^s; C = 366393/2^16 is the standard minimax bias.
SCHRAU_C1 = 184.66500888
SCHRAU_C2 = 16256.0 - 5.5907
# which k-tiles each (attn, set) offloads from ScalarE to the DVE exp
OFFLOAD = {
    (0, 0): (8,), (0, 1): (5, 11),
    (0, 2): (3, 7, 11), (0, 3): (3, 7, 11),
    (1, 0): (2, 6, 10, 14), (1, 1): (2, 6, 10, 14),
    (1, 2): (2, 6, 10, 14), (1, 3): (2, 6, 10, 13),
}


# ---------------------------------------------------------------- host packing
def _head_cols(h):
    return [j * H + h for j in range(HD)]


def _fuse_qk(wq, wk):
    """[128, H*128] bf16: col block h = M_h^T = 0.25 * wq_h @ wk_h^T."""
    out = np.empty((D, H * D), np.float32)
    for h in range(H):
        c = _head_cols(h)
        out[:, D * h : D * (h + 1)] = 0.25 * (wq[:, c] @ wk[:, c].T)
    return out


def _pack32_cols(w, grp):
    """[D, 128]: col 32g+j (j<16) = w[:, j*H + (4*grp+g)], else 0."""
    out = np.zeros((D, 128), np.float32)
    for g in range(4):
        h = 4 * grp + g
        for j in range(HD):
            out[:, 32 * g + j] = w[:, j * H + h]
    return out


def _pack_w12(w, grp):
    """lhsT for the merge projection: row 32c+j = w[j*H + (4*grp+c), :]."""
    out = np.zeros((D, D), np.float32)
    for c in range(4):
        h = 4 * grp + c
        for j in range(HD):
            out[32 * c + j, :] = w[j * H + h, :]
    return out


def _sel_matrix():
    sel = np.zeros((128, 128), np.float32)
    for m in range(128):
        sel[32 * (m // 32) + 16, m] = 1.0
    return sel


def _split_multiwaits(nc):
    """Post-pass for walrus builds that accept only ONE sync-wait per
    instruction: split every instruction carrying N>1 waits into (N-1)
    single-wait NOPs on the same engine placed immediately before it."""
    uid = 0
    for f in nc.m.functions:
        for bb in f.blocks:
            il = bb.instructions
            if not any(
                i.sync_info is not None
                and i.sync_info.on_wait
                and len(i.sync_info.on_wait) > 1
                for i in il
            ):
                continue
            out = []
            for inst in il:
                si = inst.sync_info
                if si is not None and si.on_wait and len(si.on_wait) > 1:
                    waits = list(si.on_wait)
                    for w in waits[:-1]:
                        uid += 1
                        nop = mybir.InstNoOp(
                            name=f"WSPLIT-{uid}",
                            engine=inst.engine,
                            ins=[],
                            outs=[],
                            sync_info=mybir.SyncInfo(on_wait=[w], on_update=[]),
                        )
                        out.append(nop)
                    inst.sync_info = mybir.SyncInfo(
                        on_wait=[waits[-1]], on_update=list(si.on_update)
                    )
                out.append(inst)
            bb.instructions = out
    return nc


# ---------------------------------------------------------------- device build
def build_nc():
    nc = bass.Bass()

    def din(name, shape, dt=BF16):
        return nc.dram_tensor(name, list(shape), dt, kind="ExternalInput")

    xbT = din("xbT", (128, 2048))  # batch x_tgt transposed [chan, key]
    xqT = din("xqT", (128, 512))  # query slice of x_tgt, transposed
    eoT = din("eoT", (128, 512))  # query slice of enc_out, transposed
    xqF = din("xqF", (128, 512), F32)  # fp32 copy for the residual
    mgs = din("mgs", (128, 1024))  # self-attn fused M_h^T blocks
    mgc = din("mgc", (128, 1024))  # cross-attn fused M_h^T blocks
    wv_st = din("wv_st", (D, 512))  # [v_selfA | v_selfB | v_crossA | v_crossB]
    w1p = [din(f"w1p{g}", (D, D)) for g in range(2)]
    w2p = [din(f"w2p{g}", (D, D)) for g in range(2)]
    w3 = din("w3", (D, 512))
    w4r = din("w4r", (128, 512))  # col block j = w4[128j:128j+128, :]
    selt = din("selt", (128, 128), F32)  # SEL[p, m] = (p == 32*(m//32)+16)
    onec_f = din("onec_f", (128, 1), F32)  # fp32 ones column (LN sums)
    oner_b = din("oner_b", (1, 128))  # bf16 ones row (LN b broadcast)
    c127_b = din("c127_b", (1, 128))  # bf16 127s row (LN a broadcast)
    b1t = din("b1t", (128, 1), F32)
    b2t = din("b2t", (128, 1), F32)
    b3t = din("b3t", (128, 4), F32)
    b4t = din("b4t", (128, 1), F32)
    y = nc.dram_tensor("y", [128, 512], F32, kind="ExternalOutput")

    with tile.TileContext(nc) as tc:
        with tc.tile_pool(name="persist", bufs=1) as pp, \
             tc.tile_pool(name="pattn", bufs=1, space="PSUM") as pa, \
             tc.tile_pool(name="ebp", bufs=3) as ebp:

            def sbuf(name, shape, dt=F32):
                return pp.tile(list(shape), dt, name=name, tag=name)

            def load(name, dram, shape, dt=BF16, eng=None):
                t = sbuf(name, shape, dt)
                (eng or nc.sync).dma_start(out=t[:], in_=dram[:])
                return t

            # critical loads on the SP hardware queue, ordered by first use;
            # xqT goes on the ACT queue so it lands in parallel with mgs.
            # mgs/xbT are split so the slices the first matmuls need arrive
            # as small dedicated transfers (each dma_start becomes two queue
            # entries and the scheduler interleaves all loads' halves — a
            # monolithic mgs only fully lands after every other first half).
            # critical loads on the SP hardware queue, ordered by first use;
            # xqT goes on the ACT queue so it lands in parallel with mgs.
            # Only the first G's 2-head block of mgs is split out; wv must
            # stay within the first ~4 entries (v_item(0) consumes it early).
            mgs_t = sbuf("mgs", (128, 1024), BF16)
            nc.sync.dma_start(out=mgs_t[:, :256], in_=mgs[:, :256])
            xqT_t = load("xqT", xqT, (128, 512), BF16, nc.scalar)
            xbT_t = load("xbT", xbT, (128, 2048))
            wv_t = load("wv", wv_st, (D, 512))
            nc.sync.dma_start(out=mgs_t[:, 256:], in_=mgs[:, 256:])
            mgc_t = load("mgc", mgc, (128, 1024))
            eoT_t = load("eoT", eoT, (128, 512))
            # needed by norm0/r1: ACT hardware queue (idle at t=0)
            sel_t = load("sel", selt, (128, 128), F32, nc.scalar)
            w1p_t = [load(f"w1p{g}", w1p[g], (D, D), BF16, nc.scalar)
                     for g in range(2)]
            xqF_t = load("xqF", xqF, (128, 512), F32, nc.scalar)
            b1_t = load("b1", b1t, (128, 1), F32, nc.scalar)
            b2_t = load("b2", b2t, (128, 1), F32, nc.scalar)

            v_all = sbuf("v_all", (128, KT, 512), BF16)
            g_s = [sbuf(f"gs{h}", (128, 512), BF16) for h in range(H)]
            g_c = [sbuf(f"gc{h}", (128, 512), BF16) for h in range(H)]

            # ---- setup work items, woven into the attention pipeline.
            # Items alternate between two PSUM banks (ps1/ps0, both idle
            # until the first norm) so an item's matmul never waits on the
            # previous item's PSUM->SBUF cast.
            _setup_tag = [0]

            def _setup_bank(name):
                _setup_tag[0] ^= 1
                return pa.tile([128, 512], F32, name=name,
                               tag="ps1" if _setup_tag[0] else "ps0")

            def g_item(msrc, xsrc, dst, name):
                gp = _setup_bank(name)
                nc.tensor.matmul(gp[:], lhsT=msrc, rhs=xsrc,
                                 start=True, stop=True)
                nc.vector.tensor_copy(out=dst, in_=gp[:])

            def v_item(t):
                vp = _setup_bank(f"vp{t}")
                nc.tensor.matmul(vp[:],
                                 lhsT=xbT_t[:, 128 * t : 128 * (t + 1)],
                                 rhs=wv_t[:], start=True, stop=True)
                nc.vector.tensor_copy(out=v_all[:, t, :], in_=vp[:])
                # softmax-denominator ones columns (col 16 of each 32-group)
                nc.gpsimd.memset(
                    v_all[:, t, :].rearrange("p (c x) -> p c x", x=32)[:, :, 16:17],
                    1.0,
                )

            def gs_item(h):
                return lambda: g_item(mgs_t[:, 128 * h : 128 * (h + 1)],
                                      xqT_t[:], g_s[h][:], f"gps{h}")

            def gc_item(h):
                return lambda: g_item(mgc_t[:, 128 * h : 128 * (h + 1)],
                                      eoT_t[:], g_c[h][:], f"gpc{h}")

            # G for the first set's two heads up front; everything else woven
            gs_item(0)()
            gs_item(1)()

            packed = {}  # (ai, grp) -> SBUF f32 accumulator
            rp = {}  # ai -> PSUM merge accumulator

            def attn_set(ai, st, g_heads, weave=()):
                """2 heads x 16 k-tiles, software-pipelined: scores(t) and
                exp(t) issue before PV(t-1) so the ScalarE exp stream never
                waits on PE; one optional setup item woven per tile. Tiles in
                OFFLOAD[(ai, st)] run exp on the DVE instead (Schraudolph
                bit-trick into bf16) to relieve the ScalarE bottleneck."""
                h0 = 2 * st
                off = OFFLOAD[(ai, st)]
                pv = [pa.tile([32, 512], F32, name=f"pv{ai}{st}{i}",
                              tag=f"pv{i}") for i in range(2)]
                ebs = [None] * KT
                # 2 tiles per issue group: [4 score MMs][2 exps][4 PV MMs]
                # [weave] — grouping same-type matmuls halves the ~85ns
                # PE instruction-type-transition penalty per tile.
                for g0 in range(0, KT + 2, 2):
                    for t in (g0, g0 + 1):
                        if t >= KT:
                            continue
                        sc = pa.tile([128, 1024], F32, bufs=2,
                                     name=f"sc{ai}{st}{t}", tag="sc")
                        for i in range(2):
                            nc.tensor.matmul(
                                sc[:, 512 * i : 512 * (i + 1)],
                                lhsT=xbT_t[:, 128 * t : 128 * (t + 1)],
                                rhs=g_heads[h0 + i][:],
                                start=True, stop=True,
                            )
                        if t in off:
                            # one-op Schraudolph: (sc*c1 + c2) cast to int16
                            # on the write path, bit-viewed as bf16 = ~e^sc
                            ti = ebp.tile([128, 1024], I16, name="ti",
                                          tag="ti")
                            nc.vector.tensor_scalar(
                                out=ti[:], in0=sc[:], scalar1=SCHRAU_C1,
                                scalar2=SCHRAU_C2, op0=OP.mult, op1=OP.add)
                            ebs[t] = ti[:].bitcast(BF16)
                        else:
                            eb = ebp.tile([128, 1024], BF16, name="eb",
                                          tag="eb", bufs=4)
                            nc.scalar.activation(eb[:], sc[:], AF.Exp)
                            ebs[t] = eb[:]
                    for tp in (g0 - 2, g0 - 1):
                        if not (0 <= tp < KT):
                            continue
                        for i in range(2):
                            h = h0 + i
                            v0 = 256 * ai + 128 * (h // 4) + 32 * (h % 4)
                            nc.tensor.matmul(
                                pv[i][:],
                                lhsT=v_all[:, tp, v0 : v0 + 32],
                                rhs=ebs[tp][:, 512 * i : 512 * (i + 1)],
                                start=(tp == 0), stop=(tp == KT - 1),
                                skip_group_check=True,
                            )
                    for t in (g0, g0 + 1):
                        if t >= KT:
                            continue
                        w = weave.get(t) if isinstance(weave, dict) else (
                            weave[t] if t < len(weave) else None)
                        if w is not None:
                            w()
                for i in range(2):
                    h = h0 + i
                    grp = h // 4
                    if (ai, grp) not in packed:
                        packed[(ai, grp)] = pp.tile(
                            [128, 512], F32, name=f"acc{ai}{grp}",
                            tag=f"acc{ai}{grp}")
                    nc.vector.tensor_copy(
                        out=packed[(ai, grp)][32 * (h % 4) : 32 * (h % 4) + 32, :],
                        in_=pv[i][:],
                    )

            def norm_pre(ai, grp, recip_on_act=False):
                """softmax-normalize one 4-head group (sel + 1/x + mul); the
                merge matmul is issued separately (norm_merge) a set later so
                the in-order PE queue never waits on the DVE reciprocal."""
                acc = packed[(ai, grp)]
                sbc = pa.tile([128, 512], F32, name=f"sbc{ai}{grp}",
                              tag="ps1")
                nc.tensor.matmul(sbc[:], lhsT=sel_t[:], rhs=acc[:],
                                 start=True, stop=True)
                rb = pp.tile([128, 512], F32, name=f"rb{ai}{grp}", tag="rb")
                # 1/x as exp(-ln x) on ScalarE: the 3.4us DVE iterative
                # divide would head-block the in-order DVE queue and stall
                # the offloaded-exp -> PV -> scores chain.
                lnr = pp.tile([128, 512], F32, name=f"lnr{ai}{grp}",
                              tag="lnr")
                nc.scalar.activation(lnr[:], sbc[:], AF.Ln)
                nc.scalar.activation(rb[:], lnr[:], AF.Exp, scale=-1.0)
                sn = pp.tile([128, 512], BF16, name=f"sn{ai}{grp}",
                             tag=f"sn{grp}")
                nc.vector.tensor_mul(sn[:], acc[:], rb[:])
                return sn

            def norm_merge(ai, grp, wp_t, sn):
                if ai not in rp:
                    rp[ai] = pa.tile([128, 512], F32, name=f"rp{ai}",
                                     tag="ps0")
                nc.tensor.matmul(rp[ai][:], lhsT=wp_t[grp][:], rhs=sn[:],
                                 start=(grp == 0), stop=(grp == 1),
                                 skip_group_check=True)

            # norm work is woven INTO subsequent sets (as weave items) so the
            # sel/merge matmuls never head-block the in-order PE queue while
            # waiting on DVE drains / reciprocals.
            sns = {}
            r1T = sbuf("r1T", (128, 512))
            r2T = sbuf("r2T", (128, 512))

            def npre(ai, grp, on_act=False):
                return lambda: sns.__setitem__(
                    (ai, grp), norm_pre(ai, grp, recip_on_act=on_act))

            def nmerge(ai, grp, wp_t):
                return lambda: norm_merge(ai, grp, wp_t, sns[(ai, grp)])

            def do_r1():
                # r1 = (sa@w1 + b1) + x_tgt
                nc.vector.scalar_tensor_tensor(
                    out=r1T[:], in0=rp[0][:], scalar=b1_t[:], in1=xqF_t[:],
                    op0=OP.add, op1=OP.add)

            # self-attention; setup matmuls spread thinly across all 4 sets:
            # V (needed same-tile) + the next set's G in set 0; the rest of
            # G-self in set 1; G-cross (needed from attn1) across sets 2-3.
            set0_weave = {
                t: (lambda t=t: (v_item(t), gs_item(t + 2)()) if t < 2
                    else v_item(t)) for t in range(KT)
            }
            set1_weave = {t: gs_item(t + 4) for t in range(4)}
            set2_weave = {0: gc_item(0), 1: npre(0, 0), 2: gc_item(1),
                          3: gc_item(2), 4: gc_item(3)}
            set3_weave = {0: gc_item(4), 1: nmerge(0, 0, w1p_t),
                          2: gc_item(5), 3: gc_item(6), 4: gc_item(7)}
            attn_set(0, 0, g_s, set0_weave)
            attn_set(0, 1, g_s, set1_weave)
            attn_set(0, 2, g_s, set2_weave)
            attn_set(0, 3, g_s, set3_weave)

            # remaining weights on the GpSimd SW-DGE queue (needed from
            # norm1 / tail onwards)
            w2p_t = [load(f"w2p{g}", w2p[g], (D, D), BF16, nc.gpsimd)
                     for g in range(2)]
            w3_t = load("w3", w3, (D, 512), BF16, nc.gpsimd)
            w4_t = load("w4", w4r, (128, 512), BF16, nc.gpsimd)
            onec_t = load("onec", onec_f, (128, 1), F32, nc.gpsimd)
            oner_t = load("oner", oner_b, (1, 128), BF16, nc.gpsimd)
            c127_t = load("c127", c127_b, (1, 128), BF16, nc.gpsimd)
            b3_t = load("b3", b3t, (128, 4), F32, nc.gpsimd)
            b4_t = load("b4", b4t, (128, 1), F32, nc.gpsimd)

            # cross-attention; attn0's grp1 norm folds in behind set 0
            attn_set(1, 0, g_c, {1: npre(0, 1),
                                 6: lambda: (nmerge(0, 1, w1p_t)(), do_r1())})
            attn_set(1, 1, g_c)
            attn_set(1, 2, g_c, {1: npre(1, 0), 6: nmerge(1, 0, w2p_t)})
            attn_set(1, 3, g_c)

            # dummy matmuls on idle sc slots: keep the PE's HAM clock-gate
            # at K=8/8 through the serial tail so the real LN/FFN matmuls
            # run at 2.4GHz instead of re-warming from 1.2GHz
            warm_ct = [0]

            def warm(n=2):
                for _ in range(n):
                    warm_ct[0] += 1
                    dmy = pa.tile([128, 512], F32, name=f"dmy{warm_ct[0]}",
                                  tag="sc", bufs=2)
                    nc.tensor.matmul(dmy[:], lhsT=xbT_t[:, :128],
                                     rhs=g_s[0][:], start=True, stop=True)

            warm(2)
            npre(1, 1, on_act=True)()
            warm(2)
            nmerge(1, 1, w2p_t)()
            warm(2)
            nc.vector.scalar_tensor_tensor(
                out=r2T[:], in0=rp[1][:], scalar=b2_t[:], in1=r1T[:],
                op0=OP.add, op1=OP.add)

            # ---------------- layernorm (x - m) / var, var unbiased
            sqf = sbuf("sqf", (128, 512))
            nc.vector.tensor_mul(sqf[:], r2T[:], r2T[:])
            spm = pa.tile([1, 512], F32, name="spm", tag="ps1")
            nc.tensor.matmul(spm[:], lhsT=onec_t[:], rhs=r2T[:],
                             start=True, stop=True)
            sps = pa.tile([1, 512], F32, name="sps", tag="ps0")
            nc.tensor.matmul(sps[:], lhsT=onec_t[:], rhs=sqf[:],
                             start=True, stop=True)
            warm(4)
            msb = sbuf("msb", (1, 512))
            nc.vector.tensor_copy(out=msb[:], in_=spm[:])
            # sum((x-m)^2) = ss - m^2/128  (m here = column sum)
            tm = sbuf("tm", (1, 512))
            nc.vector.scalar_tensor_tensor(
                out=tm[:], in0=msb[:], scalar=-1.0 / 128, in1=msb[:],
                op0=OP.mult, op1=OP.mult)
            tv = sbuf("tv", (1, 512))
            nc.vector.tensor_add(tv[:], sps[:], tm[:])
            # ra = 1/sum((x-m)^2); the 127 of the unbiased var is folded into
            # the broadcast lhsT (c127). 1/x via Ln->Exp on idle ScalarE.
            lnv = sbuf("lnv", (1, 512))
            nc.scalar.activation(lnv[:], tv[:], AF.Ln)
            ra = sbuf("ra", (1, 512))
            nc.scalar.activation(ra[:], lnv[:], AF.Exp, scale=-1.0)
            # b = -m/128 * 127 * ra  (m = sum/128 folded in)
            rb2 = sbuf("rb2", (1, 512))
            nc.vector.scalar_tensor_tensor(
                out=rb2[:], in0=ra[:], scalar=-127.0 / 128, in1=msb[:],
                op0=OP.mult, op1=OP.mult)
            ab_b = sbuf("ab_b", (1, 1024), BF16)
            nc.vector.tensor_copy(out=ab_b[:, :512], in_=ra[:])
            nc.vector.tensor_copy(out=ab_b[:, 512:], in_=rb2[:])
            abc = pa.tile([128, 1024], F32, name="abc", tag="sc", bufs=2)
            nc.tensor.matmul(abc[:, :512], lhsT=c127_t[:], rhs=ab_b[:, :512],
                             start=True, stop=True)
            nc.tensor.matmul(abc[:, 512:], lhsT=oner_t[:], rhs=ab_b[:, 512:],
                             start=True, stop=True)
            warm(2)
            lnf = sbuf("lnf", (128, 512))
            nc.vector.tensor_mul(lnf[:], r2T[:], abc[:, :512])
            lnT = sbuf("lnT", (128, 512), BF16)
            nc.vector.tensor_add(lnT[:], lnf[:], abc[:, 512:])

            # ---------------- FFN
            h_sb = []
            for j in range(4):
                hp = pa.tile([128, 512], F32, name=f"hp{j}",
                             tag=f"pv{j % 2}")
                nc.tensor.matmul(hp[:],
                                 lhsT=w3_t[:, 128 * j : 128 * (j + 1)],
                                 rhs=lnT[:], start=True, stop=True)
                hs = sbuf(f"hs{j}", (128, 512), BF16)
                nc.vector.tensor_scalar(
                    out=hs[:], in0=hp[:], scalar1=b3_t[:, j : j + 1],
                    scalar2=0.0, op0=OP.add, op1=OP.max,
                )
                h_sb.append(hs)
            op_ = pa.tile([128, 512], F32, name="op", tag="ps0")
            for j in range(4):
                nc.tensor.matmul(op_[:],
                                 lhsT=w4_t[:, 128 * j : 128 * (j + 1)],
                                 rhs=h_sb[j][:],
                                 start=(j == 0), stop=(j == 3),
                                 skip_group_check=True)
            oT = sbuf("oT", (128, 512))
            nc.vector.scalar_tensor_tensor(
                out=oT[:], in0=op_[:], scalar=b4_t[:], in1=r2T[:],
                op0=OP.add, op1=OP.add)
            nc.sync.dma_start(out=y[:], in_=oT[:])

    return nc


_CACHED = {}


def _get_nc():
    if "nc" not in _CACHED:
        _CACHED["nc"] = _split_multiwaits(build_nc())
    return _CACHED["nc"]


def _host_inputs(x_tgt, enc_out, self_wq, self_wk, self_wv, cross_wq, cross_wk,
                 cross_wv, w1, b1, w2, b2, w3, b3, w4, b4):
    bf = lambda a: np.ascontiguousarray(a, dtype=NPBF)
    f32 = lambda a: np.ascontiguousarray(a, dtype=np.float32)
    shared = {
        "mgs": bf(_fuse_qk(self_wq, self_wk)),
        "mgc": bf(_fuse_qk(cross_wq, cross_wk)),
        "wv_st": bf(np.concatenate(
            [_pack32_cols(self_wv, 0), _pack32_cols(self_wv, 1),
             _pack32_cols(cross_wv, 0), _pack32_cols(cross_wv, 1)], axis=1
        )),
        "w1p0": bf(_pack_w12(w1, 0)), "w1p1": bf(_pack_w12(w1, 1)),
        "w2p0": bf(_pack_w12(w2, 0)), "w2p1": bf(_pack_w12(w2, 1)),
        "w3": bf(w3),
        "w4r": bf(w4.reshape(4, 128, 128).transpose(1, 0, 2).reshape(128, 512)),
        "selt": f32(_sel_matrix()),
        "onec_f": f32(np.ones((128, 1), np.float32)),
        "oner_b": bf(np.ones((1, 128), np.float32)),
        "c127_b": bf(np.full((1, 128), 127.0, np.float32)),
        "b1t": f32(b1.reshape(128, 1)),
        "b2t": f32(b2.reshape(128, 1)),
        "b3t": f32(b3.reshape(4, 128).T),
        "b4t": f32(b4.reshape(128, 1)),
    }
    in_maps = []
    for c in range(NCORES):
        b, qb = divmod(c, 4)
        q0 = qb * QC
        im = dict(shared)
        im["xbT"] = bf(x_tgt[b].T)
        im["xqT"] = bf(x_tgt[b, q0 : q0 + QC].T)
        im["xqF"] = f32(x_tgt[b, q0 : q0 + QC].T)
        im["eoT"] = bf(enc_out[b, q0 : q0 + QC].T)
        in_maps.append(im)
    return in_maps


def _unshuf(y):
    """[128, 512] transposed output -> [512, 128] natural rows."""
    return np.ascontiguousarray(y.T)


def run_on_device(in_maps, **kw):
    nc = _get_nc()
    return run_bass_kernel_spmd(nc, in_maps, list(range(NCORES)), **kw)


def kernel(x_tgt, enc_out, self_wq, self_wk, self_wv, cross_wq, cross_wk,
           cross_wv, w1, b1, w2, b2, w3, b3, w4, b4, mask_src=None,
           mask_tgt=None, **_unused):
    args = [x_tgt, enc_out, self_wq, self_wk, self_wv, cross_wq, cross_wk,
            cross_wv, w1, b1, w2, b2, w3, b3, w4, b4]
    args = [np.asarray(a, dtype=np.float32) for a in args]
    in_maps = _host_inputs(*args)
    res = run_on_device(in_maps)
    out = np.empty((B, S, D), np.float32)
    for c in range(NCORES):
        b, qb = divmod(c, 4)
        out[b, qb * QC : (qb + 1) * QC] = _unshuf(res.results[c]["y"])
    return out
